# revision 1
# baseline (speedup 1.0000x reference)
"""ChebConv GNN (K=3, 3 layers) distributed Bass kernel for 8 NeuronCores.

kernel(**inputs) takes FULL numpy inputs (as in setup_inputs) and returns
the FULL [N, 40] float32 log_softmax output.

v2 design (matmul-scatter):
- Nodes sharded contiguously across 8 cores (12544 rows/core, 98 tiles).
- SpMM: per-edge source features gathered via dma_gather (bf16 table,
  256B payloads = node pair, parity windows keep idx in int16 range),
  then accumulated into per-dest-tile PSUM by TensorE matmuls with
  host-built one-hot-times-norm weight matrices ([128 pos, 128 row]
  bf16, streamed from HBM). No DVE slab chains, no recombine gathers.
- lo windows (table quarters 0+1) accumulate to an SBUF f32 partial;
  hi windows (quarters 2+3) accumulate in PSUM and add-drain to bf16
  tx tiles, so consumers start after half the producer AllGather.
- Dense 64x64 layers (bf16, PE transposes) + AllGather (bf16) are
  pipelined per dest quarter.
"""

import numpy as np

import concourse.bacc as bacc
import concourse.mybir as mybir
import concourse.tile as tile
from concourse.bass_utils import run_bass_kernel_spmd

C = 8
P = 128
SROWS = 12544
NT = 98
TROWS = 100352     # table rows (12544 * 8)
TPAD = TROWS + 2   # +2 so odd-parity pair windows stay in bounds
N_REAL = 100000
HID = 64
F_OUT_REAL = 40
QSIZES = [3200, 3200, 3072, 3072]
QSTART = [0, 3200, 6400, 9472]
BBASE = [0, 25600, 51200, 75776]   # table row base per quarter
QTILES = [25, 25, 24, 24]          # dest tiles per quarter
TQSTART = [0, 25, 50, 74]
WBASE = [0, 0, 51200, 51200]       # window table-row base (lo/lo/hi/hi)
WGRP = [[0, 1], [2, 3]]            # lo windows need AG q0+q1; hi need q2+q3
MAXCALL = 4096
GRP_TILES = 8                      # tiles per gather-call group
WBLK = 16                          # W chunks per streaming block

TRACE = [False]
LAST_EXEC_NS = [None]
_CACHE = {}


def _wrap_idx(idx):
    """dma_gather idx layout [128, len/16] int16: position j ->
    (partition j%16, slot j//16), replicated across 8 Q7 core groups."""
    n = len(idx)
    a = idx.astype(np.int16).reshape(n // 16, 16).T
    return np.broadcast_to(a[None], (8, 16, n // 16)).reshape(P, n // 16)


def _host_prep(edge_index, edge_attr):
    row = edge_index[0].astype(np.int64)
    col = edge_index[1].astype(np.int64)
    w = edge_attr.astype(np.float64)
    deg = np.zeros(N_REAL)
    np.add.at(deg, row, w)
    dinv = np.where(deg > 0, deg ** -0.5, 0.0)
    norm = (-(dinv[row] * w * dinv[col])).astype(np.float32)

    # source node -> table row (quarter, core, local)
    cc = col // SROWS
    jj = col % SROWS
    qb = np.searchsorted(np.cumsum(QSIZES), jj, side="right")
    tr = (np.asarray(BBASE)[qb] + cc * np.asarray(QSIZES)[qb]
          + (jj - np.asarray(QSTART)[qb]))
    win = 2 * (tr >= 51200) + (tr & 1)
    idxv = (tr - np.asarray(WBASE)[win]) >> 1

    shard = row // SROWS
    dr = row - shard * SROWS
    til = dr >> 7
    prt = dr & 127

    # chunk profile: per (tile, window), max over cores (SPMD uniform)
    cnt = np.zeros((C, NT, 4), dtype=np.int64)
    np.add.at(cnt, (shard, til, win), 1)
    prof = -(-cnt.max(axis=0) // P)   # [NT, 4] chunk counts
    assert (prof.sum(axis=1) > 0).all()

    # stream construction.
    # Gather order (positions): Q -> phase -> tile-group -> window -> tiles.
    # Matmul order (chunks):    Q -> phase -> tile-group -> tile -> window.
    gpos_of = np.full((NT, 4), -1, dtype=np.int64)   # first position of (t,w)
    segs = []          # [Q][phase] -> list of groups
    pos = 0
    for Q in range(4):
        qt = list(range(TQSTART[Q], TQSTART[Q] + QTILES[Q]))
        for phase in range(2):
            groups = []
            for g0 in range(0, len(qt), GRP_TILES):
                gt = qt[g0:g0 + GRP_TILES]
                calls = []       # (w, pos0, ni)
                for wq in WGRP[phase]:
                    s0 = pos
                    for t in gt:
                        if prof[t, wq]:
                            gpos_of[t, wq] = pos
                            pos += int(prof[t, wq]) * P
                    for cs in range(s0, pos, MAXCALL):
                        calls.append((wq, cs, min(MAXCALL, pos - cs)))
                tiles = []
                for t in gt:
                    if prof[t, WGRP[phase][0]] or prof[t, WGRP[phase][1]]:
                        tiles.append(t)
                groups.append(dict(calls=calls, tiles=tiles))
            segs.append(groups)
    totpos = pos

    # matmul-order chunk ids; W stream follows matmul order
    chunk_seq = []     # (t, w, gpos) in matmul order
    mm_of = {}         # (t, w) -> first matmul-order chunk id
    for Q in range(4):
        for phase in range(2):
            for grp in segs[Q * 2 + phase]:
                for t in grp["tiles"]:
                    for wq in WGRP[phase]:
                        if prof[t, wq]:
                            mm_of[(t, wq)] = len(chunk_seq)
                            for j in range(int(prof[t, wq])):
                                chunk_seq.append((t, wq, gpos_of[t, wq] + j * P))
    nchunks = len(chunk_seq)

    has = prof[:, [0, 1]].sum(axis=1) > 0, prof[:, [2, 3]].sum(axis=1) > 0

    # per-core gather idx + W matrices (W in matmul order)
    gidx = np.zeros((C, P, totpos // 16), dtype=np.int16)
    warr = np.zeros((C, nchunks, P, P), dtype=np.float32)
    mmbase = np.zeros((NT, 4), dtype=np.int64)
    for (t, wq), cid in mm_of.items():
        mmbase[t, wq] = cid
    order = np.lexsort((idxv, til, win, shard))
    so = {k: v[order] for k, v in dict(
        shard=shard, til=til, win=win, idx=idxv, prt=prt, norm=norm).items()}
    for c in range(C):
        lo = np.searchsorted(so["shard"], c)
        hi = np.searchsorted(so["shard"], c + 1)
        e_t, e_w = so["til"][lo:hi], so["win"][lo:hi]
        e_i, e_p, e_n = so["idx"][lo:hi], so["prt"][lo:hi], so["norm"][lo:hi]
        key = e_w * NT + e_t
        rank = np.arange(len(key)) - np.searchsorted(key, key)
        gp = gpos_of[e_t, e_w] + rank
        iv = np.zeros(totpos, dtype=np.int16)
        iv[gp] = e_i
        gidx[c] = _wrap_idx(iv)
        mc = mmbase[e_t, e_w] + (rank >> 7)
        warr[c][mc, rank & 127, e_p] = e_n

    import ml_dtypes
    warr = warr.astype(ml_dtypes.bfloat16)

    # builder metadata: for each group, per tile the chunk refs
    # (call_local_idx, gi, mm_chunk_id, start, stop)
    for Q in range(4):
        for phase in range(2):
            for grp in segs[Q * 2 + phase]:
                calls = grp["calls"]
                tinfo = []
                for t in grp["tiles"]:
                    chunks = []
                    for wq in WGRP[phase]:
                        for j in range(int(prof[t, wq])):
                            gp = gpos_of[t, wq] + j * P
                            ci = max(i for i, (_, p0, _) in enumerate(calls)
                                     if p0 <= gp)
                            gi = (gp - calls[ci][1]) // P
                            chunks.append((ci, gi, mmbase[t, wq] + j))
                    tinfo.append((t, chunks))
                grp["tiles"] = tinfo
    return dict(gidx=gidx, warr=warr, segs=segs, has_lo=has[0], has_hi=has[1],
                nchunks=nchunks, totpos=totpos)


def _win_aps(tab):
    flat = tab.ap().rearrange("a d -> (a d)")
    aps = []
    for w in range(4):
        base = WBASE[w] * HID + (w & 1) * HID
        rows = (51200 // 2) if w < 2 else ((TROWS - 51200) // 2)
        aps.append(flat[base:base + rows * 2 * HID]
                   .rearrange("(r e) -> r e", e=2 * HID))
    return aps


def _build(prep):
    f32 = mybir.dt.float32
    bf16 = mybir.dt.bfloat16
    i16 = mybir.dt.int16
    AO = mybir.AluOpType
    AF = mybir.ActivationFunctionType
    nchunks = prep["nchunks"]
    totpos = prep["totpos"]
    segs = prep["segs"]
    has_lo = prep["has_lo"]
    has_hi = prep["has_hi"]

    nc = bacc.Bacc("TRN2", target_bir_lowering=False, debug=False, num_devices=C,
                   num_swdge_queues=4)
    x_own = nc.declare_dram_parameter("x_own", [SROWS, HID], bf16, isOutput=False)
    x_table = nc.declare_dram_parameter("x_table", [TPAD, HID], bf16, isOutput=False)
    gidx_d = nc.declare_dram_parameter("gidx", [P, totpos // 16], i16, isOutput=False)
    wstr_d = nc.declare_dram_parameter("wstr", [nchunks, P, P], bf16, isOutput=False)
    Wd = nc.declare_dram_parameter("W", [3, 3, HID, HID], bf16, isOutput=False)
    bd = nc.declare_dram_parameter("b", [3, HID], f32, isOutput=False)
    yout = nc.declare_dram_parameter("yout", [SROWS, F_OUT_REAL], f32, isOutput=True)

    agin = [[nc.dram_tensor(f"agin{i}_{q}", [QSIZES[q], HID], bf16)
             for q in range(4)] for i in range(5)]
    agout = [nc.dram_tensor(f"agout{i}", [TPAD, HID], bf16, addr_space="Shared")
             for i in range(5)]

    with tile.TileContext(nc) as tc:
        with (
            tc.tile_pool(name="res", bufs=1) as res,
            tc.tile_pool(name="stage", bufs=6) as stagep,
            tc.tile_pool(name="wp", bufs=3) as wpool,
            tc.tile_pool(name="idxp", bufs=6) as idxp,
            tc.tile_pool(name="small", bufs=4) as smallp,
            tc.tile_pool(name="tt", bufs=2) as ttp,
            tc.tile_pool(name="psA", bufs=2, space="PSUM") as psA,
            tc.tile_pool(name="psB", bufs=2, space="PSUM") as psB,
            tc.tile_pool(name="txp", bufs=1) as txp,
        ):
            from concourse.masks import make_identity
            ident = res.tile([P, P], bf16)
            make_identity(nc, ident[:])
            Wt, bt = [], []
            for i in range(3):
                ws = []
                for k in range(3):
                    t = res.tile([HID, HID], bf16, tag=f"w{i}{k}")
                    nc.sync.dma_start(out=t[:], in_=Wd[i][k])
                    ws.append(t)
                Wt.append(ws)
                t = res.tile([HID, 1], f32, tag=f"bb{i}")
                nc.sync.dma_start(out=t[:], in_=bd[i][:, None])
                bt.append(t)

            tx0 = txp.tile([P, NT, HID], bf16, tag="tx0")
            tx1 = txp.tile([P, NT, HID], bf16, tag="tx1")
            tx2 = txp.tile([P, NT, HID], bf16, tag="tx2")
            part = txp.tile([P, NT, HID], f32, tag="part")
            nc.sync.dma_start(out=tx0[:], in_=x_own.ap().rearrange("(a p) d -> p a d", p=P))

            qctr = [0]
            blk_state = {"id": -1, "tile": None}

            def wblk_for(ch):
                b = ch // WBLK
                if blk_state["id"] != b:
                    nb = min(WBLK, nchunks - b * WBLK)
                    wb = wpool.tile([P, WBLK * P], bf16, tag="wb")
                    nc.sync.dma_start(
                        out=wb[:].rearrange("p (n m) -> p n m", m=P)[:, :nb, :],
                        in_=wstr_d.ap()[b * WBLK:b * WBLK + nb]
                        .rearrange("n p m -> p n m"))
                    blk_state["id"] = b
                    blk_state["tile"] = wb
                return blk_state["tile"], (ch % WBLK) * P

            def spmm(wins, out_tx, on_quarter=None):
                blk_state["id"] = -1
                for Q in range(4):
                    for phase in range(2):
                        for grp in segs[Q * 2 + phase]:
                            sts = []
                            for (wq, pos0, ni) in grp["calls"]:
                                it = idxp.tile([P, MAXCALL // 16], i16, tag="it")
                                nc.sync.dma_start(
                                    out=it[:, :ni // 16],
                                    in_=gidx_d[:, pos0 // 16:(pos0 + ni) // 16])
                                st = stagep.tile([P, MAXCALL // P, 2 * HID],
                                                 bf16, tag="st")
                                nc.gpsimd.dma_gather(
                                    st[:, :ni // P, :], wins[wq],
                                    it[:, :ni // 16], ni, ni, 2 * HID,
                                    single_packet=False,
                                    queue_num=qctr[0] % 4,
                                )
                                qctr[0] += 1
                                sts.append(st)
                            for (t, chunks) in grp["tiles"]:
                                pm = psA.tile([P, HID], f32, tag="pm")
                                for i, (ci, gi, ch) in enumerate(chunks):
                                    wb, off = wblk_for(ch)
                                    nc.tensor.matmul(
                                        pm[:], wb[:, off:off + P],
                                        sts[ci][:, gi, 0:HID],
                                        start=(i == 0),
                                        stop=(i == len(chunks) - 1))
                                if phase == 0:
                                    if has_hi[t]:
                                        nc.scalar.copy(out=part[:, t, :], in_=pm[:])
                                    else:
                                        nc.scalar.copy(out=out_tx[:, t, :], in_=pm[:])
                                else:
                                    if has_lo[t]:
                                        nc.vector.tensor_tensor(
                                            out=out_tx[:, t, :], in0=pm[:],
                                            in1=part[:, t, :], op=AO.add)
                                    else:
                                        nc.scalar.copy(out=out_tx[:, t, :], in_=pm[:])
                    if on_quarter is not None:
                        on_quarter(Q)

            def ag_quarter(src_tx, i, q):
                t0, ntq = TQSTART[q], QTILES[q]
                nc.sync.dma_start(
                    out=agin[i][q].ap().rearrange("(a p) d -> p a d", p=P),
                    in_=src_tx[:, t0:t0 + ntq, :])
                nc.gpsimd.collective_compute(
                    "AllGather", AO.bypass,
                    replica_groups=[list(range(C))],
                    ins=[agin[i][q].ap().opt()],
                    outs=[agout[i].ap()[BBASE[q]:BBASE[q] + C * QSIZES[q]].opt()],
                )

            def dense_quarter(li, q):
                # tx2 = 2*L(tx1) - tx0, then tx0 = relu(sum_k txk @ W[li][k] + b)
                t0, ntq = TQSTART[q], QTILES[q]
                nc.vector.scalar_tensor_tensor(
                    out=tx2[:, t0:t0 + ntq, :], in0=tx2[:, t0:t0 + ntq, :],
                    scalar=2.0, in1=tx0[:, t0:t0 + ntq, :],
                    op0=AO.mult, op1=AO.subtract)
                for t in range(t0, t0 + ntq):
                    tts = []
                    for xi, tx in enumerate((tx0, tx1, tx2)):
                        pt = psB.tile([HID, P], bf16, tag="tp")
                        nc.tensor.transpose(out=pt[:], in_=tx[:, t, :],
                                            identity=ident[:])
                        stt = ttp.tile([HID, P], bf16, tag=f"tt{xi}")
                        nc.scalar.copy(out=stt[:], in_=pt[:])
                        tts.append(stt)
                    pm = psB.tile([HID, P], f32, tag="mm")
                    for k in range(3):
                        nc.tensor.matmul(pm[:], Wt[li][k][:], tts[k][:],
                                         start=(k == 0), stop=(k == 2))
                    oo = ttp.tile([HID, P], bf16, tag="oo")
                    nc.scalar.activation(oo[:], pm[:], AF.Relu, bias=bt[li][:])
                    pb = psB.tile([P, HID], bf16, tag="tb")
                    nc.tensor.transpose(out=pb[:], in_=oo[:],
                                        identity=ident[:HID, :HID])
                    nc.scalar.copy(out=tx0[:, t, :], in_=pb[:])

            tabs = _win_aps(x_table)
            agi = [0]
            for li in range(3):
                is_last = li == 2
                i1 = agi[0]; agi[0] += 1
                spmm(tabs, tx1, on_quarter=lambda q, i=i1: ag_quarter(tx1, i, q))

                def after2(q, li=li, is_last=is_last):
                    dense_quarter(li, q)
                    if not is_last:
                        ag_quarter(tx0, agi[0], q)
                spmm(_win_aps(agout[i1]), tx2, on_quarter=after2)
                if not is_last:
                    i2 = agi[0]; agi[0] += 1
                    tabs = _win_aps(agout[i2])

            # log_softmax over first F_OUT_REAL features (f32)
            sh = txp.tile([P, NT, F_OUT_REAL], f32, tag="sh")
            nc.vector.tensor_copy(out=sh[:], in_=tx0[:, :, :F_OUT_REAL])
            mx = smallp.tile([P, NT, 1], f32, tag="mx")
            nc.vector.tensor_reduce(out=mx[:], in_=sh[:],
                                    axis=mybir.AxisListType.X, op=AO.max)
            nc.vector.tensor_tensor(
                out=sh[:], in0=sh[:],
                in1=mx[:].to_broadcast([P, NT, F_OUT_REAL]), op=AO.subtract)
            ex = txp.tile([P, NT, F_OUT_REAL], f32, tag="ex")
            nc.scalar.activation(ex[:], sh[:], AF.Exp)
            sm = smallp.tile([P, NT, 1], f32, tag="sm")
            nc.vector.tensor_reduce(out=sm[:], in_=ex[:],
                                    axis=mybir.AxisListType.X, op=AO.add)
            lz = smallp.tile([P, NT, 1], f32, tag="lz")
            nc.scalar.activation(lz[:], sm[:], AF.Ln)
            nc.vector.tensor_tensor(
                out=sh[:], in0=sh[:],
                in1=lz[:].to_broadcast([P, NT, F_OUT_REAL]), op=AO.subtract)
            nc.sync.dma_start(
                out=yout.ap().rearrange("(a p) d -> p a d", p=P), in_=sh[:])
    nc.compile()
    return nc


def kernel(x, edge_index, edge_attr, W0, b0, W1, b1, W2, b2):
    import ml_dtypes
    x = np.asarray(x)
    edge_index = np.asarray(edge_index)
    edge_attr = np.asarray(edge_attr)
    key = hash((edge_index.tobytes(), edge_attr.tobytes()))
    if key in _CACHE:
        nc, prep = _CACHE[key]
    else:
        prep = _host_prep(edge_index, edge_attr)
        nc = _build(prep)
        _CACHE[key] = (nc, prep)

    W2p = np.zeros((3, HID, HID), dtype=np.float32)
    W2p[:, :, :F_OUT_REAL] = np.asarray(W2, dtype=np.float32)
    b2p = np.zeros((HID,), dtype=np.float32)
    b2p[:F_OUT_REAL] = np.asarray(b2, dtype=np.float32)
    Wall = np.stack([np.asarray(W0, np.float32), np.asarray(W1, np.float32),
                     W2p]).astype(ml_dtypes.bfloat16)
    ball = np.stack([np.asarray(b0, np.float32), np.asarray(b1, np.float32),
                     b2p]).astype(np.float32)

    xpad = np.zeros((TROWS, HID), dtype=np.float32)
    xpad[:N_REAL] = np.asarray(x, dtype=np.float32)
    xtab = np.zeros((TPAD, HID), dtype=ml_dtypes.bfloat16)
    for q in range(4):
        for c in range(C):
            src0 = c * SROWS + QSTART[q]
            dst0 = BBASE[q] + c * QSIZES[q]
            xtab[dst0:dst0 + QSIZES[q]] = xpad[src0:src0 + QSIZES[q]].astype(
                ml_dtypes.bfloat16)

    in_maps = []
    for c in range(C):
        in_maps.append({
            "x_own": xpad[c * SROWS:(c + 1) * SROWS].astype(ml_dtypes.bfloat16),
            "x_table": xtab,
            "gidx": prep["gidx"][c],
            "wstr": prep["warr"][c],
            "W": Wall, "b": ball,
        })
    res = run_bass_kernel_spmd(nc, in_maps, core_ids=list(range(C)),
                               trace=TRACE[0])
    LAST_EXEC_NS[0] = res.exec_time_ns
    out = np.concatenate([res.results[c]["yout"] for c in range(C)], axis=0)
    return out[:N_REAL].astype(np.float32)

